# revision 1
# baseline (speedup 1.0000x reference)
"""Causal depthwise conv1d (K=4) over packed ragged sequences + SiLU + conv-state
cache update, sharded channel-wise across 8 trn2 NeuronCores.

Strategy:
  - Channels (D=4096) sharded 512/core (tensor-parallel, per the module's tp logic).
  - Host transposes x to channel-major (D, T) so each core DMAs contiguous rows;
    on-chip layout is [channels->partitions, tokens->free], so conv taps are just
    free-dim offsets.
  - The depthwise conv runs on the TensorEngine: tap j is a matmul with a
    diagonal stationary matrix diag(w[:, j]) accumulating into PSUM (4 taps ->
    4 matmuls into one PSUM bank), then one ScalarEngine pass computes
    silu(acc + bias) and writes the output tile.
  - Sequence-boundary tokens (first 3 tokens of each sequence, <= 27 rows total)
    are recomputed exactly on the host afterwards; the conv-state cache update
    (1MB gather/scatter) is also metadata-sized and done on the host.
"""

import numpy as np

T = 16384
D = 4096
K = 4
NCORES = 8
DC = D // NCORES  # 512 channels per core
G = DC // 128     # 4 partition groups per core
F = 512           # token tile (one fp32 PSUM bank)
NT = T // F       # 32 token tiles
HALO = K - 1      # 3

_cached_nc = None


def _build_device_kernel():
    import concourse.bacc as bacc
    import concourse.mybir as mybir
    from concourse.tile import TileContext

    f32 = mybir.dt.float32
    nc = bacc.Bacc("TRN2", target_bir_lowering=False, debug=False,
                   num_devices=NCORES)

    xt = nc.dram_tensor("xt", [DC, T], f32, kind="ExternalInput")
    wd = nc.dram_tensor("wd", [G, K, 128, 128], f32, kind="ExternalInput")
    bs = nc.dram_tensor("bs", [128, G], f32, kind="ExternalInput")
    yt = nc.dram_tensor("yt", [DC, T], f32, kind="ExternalOutput")

    with TileContext(nc) as tc:
        with (
            tc.tile_pool(name="const", bufs=1) as cpool,
            tc.tile_pool(name="xp", bufs=6) as xpool,
            tc.tile_pool(name="yp", bufs=6) as ypool,
            tc.tile_pool(name="ps", bufs=4, space="PSUM") as ppool,
        ):
            wd_sb = cpool.tile([128, G, K, 128], f32)
            nc.sync.dma_start(out=wd_sb[:], in_=wd.rearrange("g k p f -> p g k f"))
            bs_sb = cpool.tile([128, G], f32)
            nc.sync.dma_start(out=bs_sb[:], in_=bs[:])

            for g in range(G):
                rows = slice(g * 128, (g + 1) * 128)
                for i in range(NT):
                    xtile = xpool.tile([128, F + HALO], f32)
                    if i == 0:
                        nc.gpsimd.memset(xtile[:, 0:HALO], 0.0)
                        nc.sync.dma_start(out=xtile[:, HALO:], in_=xt[rows, 0:F])
                    else:
                        nc.sync.dma_start(
                            out=xtile[:], in_=xt[rows, i * F - HALO:(i + 1) * F]
                        )
                    ps = ppool.tile([128, F], f32)
                    for j in range(K):
                        # tap j: out[c, t] += w[c, j] * x[c, t - (K-1-j)]
                        nc.tensor.matmul(
                            ps[:],
                            wd_sb[:, g, j, :],
                            xtile[:, j:j + F],
                            start=(j == 0),
                            stop=(j == K - 1),
                        )
                    ytile = ypool.tile([128, F], f32)
                    nc.scalar.activation(
                        ytile[:], ps[:], mybir.ActivationFunctionType.Silu,
                        bias=bs_sb[:, g:g + 1], scale=1.0,
                    )
                    nc.sync.dma_start(out=yt[rows, i * F:(i + 1) * F], in_=ytile[:])

    nc.compile()
    return nc


def _get_nc():
    global _cached_nc
    if _cached_nc is None:
        _cached_nc = _build_device_kernel()
    return _cached_nc


def _silu(a):
    return a * (1.0 / (1.0 + np.exp(-a)))


def kernel(x, weight, bias, conv_state, seq_idx, conv_idx, state_ids,
           _run_opts=None):
    from concourse.bass_utils import run_bass_kernel_spmd

    x = np.asarray(x)
    weight = np.asarray(weight)
    bias = np.asarray(bias)
    conv_state = np.asarray(conv_state)
    seq_idx = np.asarray(seq_idx)
    conv_idx = np.asarray(conv_idx)
    state_ids = np.asarray(state_ids)

    x0 = x[0]                              # (T, D) f32
    w = weight[:, 0, :].astype(np.float32)  # (D, K)
    xT = np.ascontiguousarray(x0.T)        # (D, T)

    in_maps = []
    for c in range(NCORES):
        lo = c * DC
        w_core = w[lo:lo + DC]             # (DC, K)
        wd = np.zeros((G, K, 128, 128), dtype=np.float32)
        for g in range(G):
            for j in range(K):
                np.fill_diagonal(wd[g, j], w_core[g * 128:(g + 1) * 128, j])
        bs = np.ascontiguousarray(
            bias[lo:lo + DC].astype(np.float32).reshape(G, 128).T
        )                                   # (128, G)
        in_maps.append({
            "xt": np.ascontiguousarray(xT[lo:lo + DC]),
            "wd": wd,
            "bs": bs,
        })

    nc = _get_nc()
    run_opts = _run_opts or {}
    res = run_bass_kernel_spmd(nc, in_maps, core_ids=list(range(NCORES)),
                               **run_opts)

    outT = np.concatenate([r["yt"] for r in res.results], axis=0)  # (D, T)
    out = np.ascontiguousarray(outT.T)[None]                       # (1, T, D)

    # --- host fixup: first K-1 tokens of every sequence (exact recompute) ---
    starts = np.concatenate([[0], np.flatnonzero(np.diff(seq_idx) != 0) + 1])
    fix = (starts[:, None] + np.arange(HALO)[None]).ravel()
    fix = np.unique(fix[fix < T])
    if fix.size:
        acc = np.broadcast_to(bias.astype(np.float32), (fix.size, D)).copy()
        for j in range(K):
            s = K - 1 - j
            tm = fix - s
            tm_c = np.clip(tm, 0, T - 1)
            valid = (tm >= 0) & (seq_idx[tm_c] == seq_idx[fix])
            acc += np.where(valid[:, None], x0[tm_c], 0.0) * w[None, :, j]
        out[0, fix] = _silu(acc)

    # --- conv-state cache update (gather last-K rows, scatter into pool) ---
    new_conv_state = conv_state.copy()
    new_conv_state[state_ids] = np.transpose(x0[conv_idx], (0, 2, 1))

    if _run_opts is not None:
        return (out, new_conv_state), res
    return out, new_conv_state


# revision 2
# speedup vs baseline: 1.9591x; 1.9591x over previous
"""Causal depthwise conv1d (K=4) over packed ragged sequences + SiLU + conv-state
cache update, sharded channel-wise across 8 trn2 NeuronCores.

Strategy:
  - Channels (D=4096) sharded 512/core (tensor-parallel, per the module's tp
    logic). Host transposes x to channel-major (D, T) so each core DMAs
    contiguous rows; on-chip layout is [channels->partitions, tokens->free],
    so conv taps are free-dim offsets.
  - fp32 matmul on the PE runs at ~4 cyc/col (2-pass HI/LO) and fp32
    tensor-tensor on the DVE runs at 1 elem/lane/cyc, so neither engine alone
    covers 4 taps under the HBM roofline. Work is split by token tile:
      * PE tiles (F=512): 4 diagonal-matrix matmuls accumulate the taps in
        PSUM (per-channel scale = diagonal stationary).
      * DVE tiles (F=2048): tensor_scalar (2x mode) + 3 scalar_tensor_tensor
        fused MACs.
    ScalarE runs silu(acc + bias) for every tile.
  - Sequence-boundary tokens (first 3 of each sequence, <= 27 rows) are
    recomputed exactly on the host; the conv-state cache update (1MB
    gather/scatter) is also metadata-sized and done on the host.
"""

import numpy as np

T = 16384
D = 4096
K = 4
NCORES = 8
DC = D // NCORES  # 512 channels per core
G = DC // 128     # 4 partition groups per core
HALO = K - 1      # 3

F_PE = 512        # PE tile (one fp32 PSUM bank)
F_DVE = 2048      # DVE tile
# per 6 slots of 512 tokens: 2 PE tiles + 1 DVE tile (4 slots)
PE_SLOTS_PER_PERIOD = 2
PERIOD = PE_SLOTS_PER_PERIOD + F_DVE // F_PE  # 6
NSLOT = T // F_PE  # 32

_cached_nc = None


def _schedule():
    """Per-group work list: [('pe'|'dve', t0), ...] covering T tokens."""
    items = []
    t = 0
    while t < T:
        rem = (T - t) // F_PE
        if rem >= PERIOD:
            for _ in range(PE_SLOTS_PER_PERIOD):
                items.append(("pe", t))
                t += F_PE
            items.append(("dve", t))
            t += F_DVE
        else:
            items.append(("pe", t))
            t += F_PE
    return items


def _build_device_kernel():
    import concourse.bacc as bacc
    import concourse.mybir as mybir
    from concourse.tile import TileContext

    f32 = mybir.dt.float32
    mult = mybir.AluOpType.mult
    add = mybir.AluOpType.add
    silu_fn = mybir.ActivationFunctionType.Silu

    nc = bacc.Bacc("TRN2", target_bir_lowering=False, debug=False,
                   num_devices=NCORES)

    xt = nc.dram_tensor("xt", [DC, T], f32, kind="ExternalInput")
    wd = nc.dram_tensor("wd", [G, K, 128, 128], f32, kind="ExternalInput")
    ws = nc.dram_tensor("ws", [128, G, K], f32, kind="ExternalInput")
    bs = nc.dram_tensor("bs", [128, G], f32, kind="ExternalInput")
    yt = nc.dram_tensor("yt", [DC, T], f32, kind="ExternalOutput")

    sched = _schedule()

    with TileContext(nc) as tc:
        with (
            tc.tile_pool(name="const", bufs=1) as cpool,
            tc.tile_pool(name="xpe", bufs=4) as xpe_pool,
            tc.tile_pool(name="ype", bufs=4) as ype_pool,
            tc.tile_pool(name="xdv", bufs=3) as xdv_pool,
            tc.tile_pool(name="tdv", bufs=3) as tdv_pool,
            tc.tile_pool(name="ps", bufs=4, space="PSUM") as ppool,
        ):
            wd_sb = cpool.tile([128, G, K, 128], f32)
            nc.sync.dma_start(out=wd_sb[:], in_=wd.rearrange("g k p f -> p g k f"))
            ws_sb = cpool.tile([128, G, K], f32)
            nc.sync.dma_start(out=ws_sb[:], in_=ws[:])
            bs_sb = cpool.tile([128, G], f32)
            nc.sync.dma_start(out=bs_sb[:], in_=bs[:])

            for g in range(G):
                rows = slice(g * 128, (g + 1) * 128)
                for kind, t0 in sched:
                    F = F_PE if kind == "pe" else F_DVE
                    xtile = (xpe_pool if kind == "pe" else xdv_pool).tile(
                        [128, F + HALO], f32)
                    if t0 == 0:
                        nc.gpsimd.memset(xtile[:, 0:HALO], 0.0)
                        nc.sync.dma_start(out=xtile[:, HALO:], in_=xt[rows, 0:F])
                    else:
                        nc.sync.dma_start(
                            out=xtile[:], in_=xt[rows, t0 - HALO:t0 + F])

                    if kind == "pe":
                        ps = ppool.tile([128, F], f32)
                        for j in range(K):
                            # tap j: out[c,t] += w[c,j] * x[c, t-(K-1-j)]
                            nc.tensor.matmul(
                                ps[:], wd_sb[:, g, j, :], xtile[:, j:j + F],
                                start=(j == 0), stop=(j == K - 1),
                            )
                        ytile = ype_pool.tile([128, F], f32)
                        nc.scalar.activation(
                            ytile[:], ps[:], silu_fn,
                            bias=bs_sb[:, g:g + 1], scale=1.0)
                        nc.sync.dma_start(out=yt[rows, t0:t0 + F], in_=ytile[:])
                    else:
                        acc = tdv_pool.tile([128, F], f32)
                        nc.vector.tensor_scalar_mul(
                            acc[:], xtile[:, 0:F], ws_sb[:, g, 0:1])
                        for j in range(1, K):
                            nc.vector.scalar_tensor_tensor(
                                out=acc[:], in0=xtile[:, j:j + F],
                                scalar=ws_sb[:, g, j:j + 1], in1=acc[:],
                                op0=mult, op1=add)
                        nc.scalar.activation(
                            acc[:], acc[:], silu_fn,
                            bias=bs_sb[:, g:g + 1], scale=1.0)
                        nc.sync.dma_start(out=yt[rows, t0:t0 + F], in_=acc[:])

    nc.compile()
    return nc


def _get_nc():
    global _cached_nc
    if _cached_nc is None:
        _cached_nc = _build_device_kernel()
    return _cached_nc


def _silu(a):
    return a * (1.0 / (1.0 + np.exp(-a)))


def kernel(x, weight, bias, conv_state, seq_idx, conv_idx, state_ids,
           _run_opts=None):
    from concourse.bass_utils import run_bass_kernel_spmd

    x = np.asarray(x)
    weight = np.asarray(weight)
    bias = np.asarray(bias)
    conv_state = np.asarray(conv_state)
    seq_idx = np.asarray(seq_idx)
    conv_idx = np.asarray(conv_idx)
    state_ids = np.asarray(state_ids)

    x0 = x[0]                               # (T, D) f32
    w = weight[:, 0, :].astype(np.float32)  # (D, K)
    xT = np.ascontiguousarray(x0.T)         # (D, T)

    in_maps = []
    for c in range(NCORES):
        lo = c * DC
        w_core = w[lo:lo + DC]              # (DC, K)
        wdm = np.zeros((G, K, 128, 128), dtype=np.float32)
        for g in range(G):
            for j in range(K):
                np.fill_diagonal(wdm[g, j], w_core[g * 128:(g + 1) * 128, j])
        wsm = np.ascontiguousarray(
            w_core.reshape(G, 128, K).transpose(1, 0, 2))   # (128, G, K)
        bsm = np.ascontiguousarray(
            bias[lo:lo + DC].astype(np.float32).reshape(G, 128).T)  # (128, G)
        in_maps.append({
            "xt": np.ascontiguousarray(xT[lo:lo + DC]),
            "wd": wdm,
            "ws": wsm,
            "bs": bsm,
        })

    nc = _get_nc()
    run_opts = _run_opts or {}
    res = run_bass_kernel_spmd(nc, in_maps, core_ids=list(range(NCORES)),
                               **run_opts)

    outT = np.concatenate([r["yt"] for r in res.results], axis=0)  # (D, T)
    out = np.ascontiguousarray(outT.T)[None]                       # (1, T, D)

    # --- host fixup: first K-1 tokens of every sequence (exact recompute) ---
    starts = np.concatenate([[0], np.flatnonzero(np.diff(seq_idx) != 0) + 1])
    fix = (starts[:, None] + np.arange(HALO)[None]).ravel()
    fix = np.unique(fix[fix < T])
    if fix.size:
        acc = np.broadcast_to(bias.astype(np.float32), (fix.size, D)).copy()
        for j in range(K):
            s = K - 1 - j
            tm = fix - s
            tm_c = np.clip(tm, 0, T - 1)
            valid = (tm >= 0) & (seq_idx[tm_c] == seq_idx[fix])
            acc += np.where(valid[:, None], x0[tm_c], 0.0) * w[None, :, j]
        out[0, fix] = _silu(acc)

    # --- conv-state cache update (gather last-K rows, scatter into pool) ---
    new_conv_state = conv_state.copy()
    new_conv_state[state_ids] = np.transpose(x0[conv_idx], (0, 2, 1))

    if _run_opts is not None:
        return (out, new_conv_state), res
    return out, new_conv_state
